# revision 3
# baseline (speedup 1.0000x reference)
r"""Boson-sampling probability |Perm(A)|^2 via Glynn's formula on 8 Trainium2 cores.

Math
----
perm(A) = 2^(1-n) * sum_{d in {-1,+1}^n} (prod_i d_i) * prod_j (sum_i d_i A_ij), n=20.
Terms for d and -d are equal, so enumerate d_19 = -1 only and double.

Sign-bit allocation for the remaining 19 bits:
  bits 0..8   -> free axis f (512)       [same on every core]
  bits 9..15  -> partition axis p (128)  [same on every core]
  bits 16..18 -> core c (8)

Row vector V_j(p,f,c) = Cp_c[p,j] + Cf[f,j] with
  Cp_c[p,j] = sum_{i=9..15} d_i(p) A[i,j] + sum_{i=16..18} d_i(c) A[i,j] - A[19,j]
  Cf[f,j]   = sum_{i=0..8} d_i(f) A[i,j]

Split the j-product into groups GA=0..6, GB=7..13, GC=14..19. Each group
product expands over subsets T of the group:
  PG[p,f] = sum_T (prod_{j in T} Cp[p,j]) * (prod_{j in G\T} Cf[f,j])
a bilinear form of rank 2^|G| -> computed on TensorE as fp16 matmuls with
PSUM accumulation. VectorE combines P = PA*PB*PC (complex) and reduces
over f with fused scalar_tensor_tensor accumulate ops; the (128,4)
per-core partials are summed on host in float64.

Schedule (from perfetto analysis; v1 @23.5us -> this @~22.1us measured):
 - Table loads spread over all three DGE paths so the SDMA queues
   round-robin at full aggregate share under 8-core HBM contention
   (~265 GB/s/core active streaming): B (chain gate) alone on the SP
   HWDGE ring, C then A0 on the ACT ring, A1 via GPSIMD SWDGE. One
   packed [128, 5*768] fp16 DRAM tensor per core, partition-contiguous,
   one descriptor per partition per transfer.
 - 26 F=256 dummy matmuls bridge the PE HAM activity window across the
   ~6us DMA wait, so the real matmuls run at the ramped clock
   (216ns/512-col matmul instead of 427/585).
 - Evictions split ACT/DVE: ACT copies sPCre (after the Cre matmul
   alone), sPCim, sPBim; the DVE casts sPBre itself so the combine
   chain launches the moment the Bre accumulation closes. DVE is the
   critical-path engine: cast + 4 muls + sub/add + 4 fused-reduce STTs
   (~5.0us busy, all 1x/2x bound).
 - ACT warms its activation table with a dummy copy at t=0 so the
   1.3us ACT_TABLE_LOAD overlaps the DMA head.
 - The PE partition-reduces the four [128,1] accumulator columns with a
   ones-lhsT matmul so the output store is a single 16B descriptor.
 - Fixed, non-removable overhead measured on this runtime: ~1.2us bass
   init barrier + ~0.7us descgen + ~0.8us doorbell-to-first-byte +
   ~6.7us end-of-NEFF semaphore-ceremony (53 EVENT_SEMAPHOREs/engine in
   lockstep, count invariant to kernel semaphore usage) + ~1.4us output
   DMA round trip.
"""

import numpy as np

N = 20
N_CORES = 8
F = 512           # free size (bits 0..8)
P = 128           # partitions (bits 9..15)
GA = list(range(0, 7))
GB = list(range(7, 14))
GC = list(range(14, 20))
WCH = 2 * P + F   # 768 columns per chunk: [lhsT_re | lhsT_im | vtab]
# chunk order in the packed table / SBUF: C(1), B(2), A(2)
NCH = 5

_PROGRAM_CACHE = {}


def _signs(count, nbits):
    v = np.arange(count, dtype=np.int64)[:, None]
    return (((v >> np.arange(nbits)) & 1) * 2.0 - 1.0)  # (count, nbits) float64


def _subset_prods(C):
    """C: (nvals, g) complex128 -> (2^g, nvals); row T = prod_{k: bit k of T} C[:, k]."""
    out = np.ones((1, C.shape[0]), np.complex128)
    for k in range(C.shape[1]):
        out = np.concatenate([out, out * C[None, :, k]], axis=0)
    return out


def _pack_group(U, V):
    """Interleave re/im rows for the paired-contraction matmul layout.

    One shared V table streams through two matmuls; the re/im arithmetic is
    carried by two lhsT variants (contraction rows m = 2T + c):
      vtab[2T]   = Re V[T],  vtab[2T+1]   = Im V[T]
      lhs_re[2T] = Re U[T],  lhs_re[2T+1] = -Im U[T]   (-> PG_re)
      lhs_im[2T] = Im U[T],  lhs_im[2T+1] =  Re U[T]   (-> PG_im)
    """
    nT = U.shape[0]
    lre = np.empty((2 * nT, U.shape[1]), np.float32)
    lre[0::2] = U.real
    lre[1::2] = -U.imag
    lim = np.empty((2 * nT, U.shape[1]), np.float32)
    lim[0::2] = U.imag
    lim[1::2] = U.real
    vtab = np.empty((2 * nT, V.shape[1]), np.float32)
    vtab[0::2] = V.real
    vtab[1::2] = V.imag
    return lre, lim, vtab


def _build_core_tables(A, core):
    """Host tables for one core, packed as one (128, 5*768) fp16 array with
    per-partition-contiguous chunk layout [C | B0 | B1 | A0 | A1]."""
    f_signs = _signs(F, 9)
    p_signs = _signs(P, 7)
    c_signs = _signs(N_CORES, 3)
    par_f = np.prod(f_signs, axis=1)
    par_p = np.prod(p_signs, axis=1)
    par_c = np.prod(c_signs[core])

    Cf = f_signs @ A[0:9, :]                                         # (512, 20)
    Cp = p_signs @ A[9:16, :] + (c_signs[core] @ A[16:19, :] - A[19, :])[None, :]

    chunks = {}
    for name, G in (("A", GA), ("B", GB), ("C", GC)):
        U = _subset_prods(Cp[:, G])          # (2^g, 128)
        VV = _subset_prods(Cf[:, G])         # (2^g, 512)
        V = VV[::-1]                         # complement subset: T -> 2^g-1-T
        if name == "A":
            # fold full parity: par_p(p) * par_f(f) * par_c * (-1 for d19)
            U = U * (par_p[None, :] * (-par_c))
            V = V * par_f[None, :]
        lre, lim, vtab = _pack_group(U, V)
        nchunks = lre.shape[0] // 128
        packed = np.concatenate([lre, lim, vtab], axis=1)  # (2^g*2, 768)
        chunks[name] = packed.reshape(nchunks, 128, WCH).astype(np.float16)
    # partition-major packing: [B0 B1 | C | A0 A1]; B rides the SP ring
    # (chain gate), C the ACT ring (small, early), A0/A1 ACT+GP (needed last)
    tab = np.concatenate(
        [chunks["B"][0], chunks["B"][1], chunks["C"][0],
         chunks["A"][0], chunks["A"][1]], axis=1)          # (128, 3840)
    return {"tab": np.ascontiguousarray(tab)}


def _build_program():
    if "prog" in _PROGRAM_CACHE:
        return _PROGRAM_CACHE["prog"]

    from contextlib import ExitStack
    from concourse import bass, mybir

    f32 = mybir.dt.float32
    f16 = mybir.dt.float16
    mul = mybir.AluOpType.mult
    nc = bass.Bass()

    W = NCH * WCH
    tab_dram = nc.declare_dram_parameter("tab", [128, W], f16, isOutput=False)
    out_dram = nc.declare_dram_parameter("out", [1, 4], f32, isOutput=True)

    es = ExitStack()
    with es:
        dsem = [es.enter_context(nc.semaphore(f"dma{i}")) for i in range(4)]
        pe_sem = es.enter_context(nc.semaphore("pe_sem"))
        act_sem = es.enter_context(nc.semaphore("act_sem"))
        dve_sem = es.enter_context(nc.semaphore("dve_sem"))
        gp_sem = es.enter_context(nc.semaphore("gp_sem"))

        sb = es.enter_context(nc.sbuf_tensor("sb_tab", [128, W], f16))
        names = ["sPCre", "sPCim", "sPBre", "sPBim",
                 "t1", "t2", "t3", "t4", "U_", "W_",
                 "scr1", "scr2", "scr3", "scr4"]
        wt = {n: es.enter_context(nc.sbuf_tensor(n, [P, F], f16)) for n in names}
        out_t = es.enter_context(nc.sbuf_tensor("out_t", [P, 4], f32))
        out_s = es.enter_context(nc.sbuf_tensor("out_s", [1, 4], f32))
        dummy = es.enter_context(nc.sbuf_tensor("actwarm", [P, 2], f32))
        pewarm = es.enter_context(nc.sbuf_tensor("pewarm", [P, 3 * P], f16))
        ones_ap = nc.const_aps.aps[(f32, 1.0)]
        pg = {}
        for g in ("A", "B", "C"):
            for comp in ("re", "im"):
                pg[g + comp] = es.enter_context(
                    nc.psum_tensor("pg" + g + comp, [P, F], f32))
        pgwarm = es.enter_context(nc.psum_tensor("pgwarm", [P, F], f32))

        # chunk column offsets in sb/tab: C, B0, B1, A0, A1
        def ap_lhs(ci, comp):
            lo = ci * WCH + (0 if comp == "re" else P)
            return sb[:, lo:lo + P]

        def ap_rhs(ci):
            lo = ci * WCH + 2 * P
            return sb[:, lo:lo + F]

        def _gp_stream():
            gp = nc.gpsimd
            # zero the PE-warmup operand region so the dummy matmuls can't
            # stream NaN bit patterns
            gp.memset(pewarm[:, :], 0.0).then_inc(gp_sem, 1)
            # A1 via SWDGE (needed last; Pool descgen runs off-critical-path)
            gp.dma_start(sb[:, 4 * WCH:5 * WCH], tab_dram[:, 4 * WCH:5 * WCH]).then_inc(dsem[3], 16)

        def _sync_stream():
            sync = nc.sync
            # B (chain gate) alone on the SP ring: earliest big transfer
            sync.dma_start(sb[:, 0:2 * WCH], tab_dram[:, 0:2 * WCH]).then_inc(dsem[0], 16)

        def _pe_stream():
            pe = nc.tensor
            # Dummy matmuls during the DMA wait keep the PE HAM activity
            # window full, so the real matmuls run at the 2.4GHz p-state
            # (213ns per F=512 matmul) instead of 1.2GHz.
            pe.wait_ge(gp_sem, 1)
            # F=256 dummies bridge the PE HAM activity window across the DMA
            # wait, so the real matmuls run at the ramped clock. On the
            # slowest core the tables arrive ~6us in; two dozen dummies keep
            # the PE busy until then without delaying an early arrival much.
            for _ in range(26):
                pe.matmul(pgwarm[:, 0:2 * P], pewarm[:, 0:P], pewarm[:, P:3 * P],
                          start=True, stop=True)

            # pe_sem: PC done at 2, PBre at 4, PBim at 6, PAre at 8, PAim at 10
            # chunk order in sb: B0=0, B1=1, C=2, A0=3, A1=4
            def mm(g, comp, ci, first, last, w=None):
                if w is not None:
                    pe.wait_ge(dsem[w], 16)
                pe.matmul(
                    pg[g + comp][:, :], ap_lhs(ci, comp), ap_rhs(ci),
                    start=first, stop=last,
                ).then_inc(pe_sem, 1)
            mm("C", "re", 2, True, True, w=1)
            mm("C", "im", 2, True, True)
            mm("B", "re", 0, True, False, w=0)
            mm("B", "re", 1, False, True)
            mm("B", "im", 0, True, False)
            mm("B", "im", 1, False, True)
            mm("A", "re", 3, True, False, w=2)
            mm("A", "re", 4, False, True, w=3)
            mm("A", "im", 3, True, False)
            mm("A", "im", 4, False, True)
            # partition-reduce the 4 accumulator columns so the store is one
            # 16B descriptor instead of 128
            pe.wait_ge(dve_sem, 10)
            pe.matmul(pgwarm[0:1, 0:4], ones_ap, out_t[:, :],
                      start=True, stop=True).then_inc(pe_sem, 1)

        def _act_stream():
            act = nc.scalar
            # C (small, needed first) then A0 on the ACT HWDGE ring
            act.dma_start(sb[:, 2 * WCH:3 * WCH], tab_dram[:, 2 * WCH:3 * WCH]).then_inc(dsem[1], 16)
            act.dma_start(sb[:, 3 * WCH:4 * WCH], tab_dram[:, 3 * WCH:4 * WCH]).then_inc(dsem[2], 16)
            # dummy first op: pulls the 1.3us ACT table load into the DMA head
            act.copy(dummy[:, 1:2], dummy[:, 0:1])
            # PSUM->SBUF fp16 evictions (PC, PBim); DVE casts sPBre itself.
            # sPCre only needs the Cre matmul (pe>=1), so it starts while
            # Cim is still streaming.
            act.wait_ge(pe_sem, 1)
            act.copy(wt["sPCre"][:, :], pg["Cre"][:, :]).then_inc(act_sem, 1)
            act.wait_ge(pe_sem, 2)
            act.copy(wt["sPCim"][:, :], pg["Cim"][:, :]).then_inc(act_sem, 1)
            act.wait_ge(pe_sem, 6)
            act.copy(wt["sPBim"][:, :], pg["Bim"][:, :]).then_inc(act_sem, 1)
            # evict the PE's 4-column partition-reduction, store via a single
            # 16B descriptor
            act.wait_ge(pe_sem, 11)
            act.copy(out_s[:, :], pgwarm[0:1, 0:4])
            act.dma_start(out_dram[:], out_s[:, :]).then_inc(dsem[0], 16)

        def _dve_stream():
            v = nc.vector
            # M = PC*PB in fp16 2x mode; fused 1x dot-products against PA in
            # PSUM. Standalone self-waits make same-engine RAW explicit.
            v.memset(dummy[:, 0:1], 0.0)
            # self-evict PBre (2x-mode fp16 cast) so the chain isn't queued
            # behind ACT's eviction backlog
            v.wait_ge(pe_sem, 4)
            v.tensor_copy(wt["sPBre"][:, :], pg["Bre"][:, :])
            v.wait_ge(act_sem, 1)
            v.tensor_mul(wt["t1"][:, :], wt["sPCre"][:, :], wt["sPBre"][:, :]).then_inc(dve_sem, 1)
            v.wait_ge(act_sem, 2)
            v.tensor_mul(wt["t4"][:, :], wt["sPCim"][:, :], wt["sPBre"][:, :]).then_inc(dve_sem, 1)
            v.wait_ge(act_sem, 3)
            v.tensor_mul(wt["t2"][:, :], wt["sPCim"][:, :], wt["sPBim"][:, :]).then_inc(dve_sem, 1)
            v.tensor_mul(wt["t3"][:, :], wt["sPCre"][:, :], wt["sPBim"][:, :]).then_inc(dve_sem, 1)
            v.wait_ge(dve_sem, 3)
            v.tensor_sub(wt["U_"][:, :], wt["t1"][:, :], wt["t2"][:, :]).then_inc(dve_sem, 1)
            v.wait_ge(dve_sem, 4)
            v.tensor_add(wt["W_"][:, :], wt["t3"][:, :], wt["t4"][:, :]).then_inc(dve_sem, 1)
            # out cols: 0 = sum U*PAre, 1 = sum W*PAim, 2 = sum U*PAim,
            # 3 = sum W*PAre ; host computes re = c0-c1, im = c2+c3.
            v.wait_ge(pe_sem, 8)
            v.wait_ge(dve_sem, 5)
            v.scalar_tensor_tensor(
                wt["scr1"][:, :], wt["U_"][:, :], 1.0, pg["Are"][:, :],
                mul, mul, accum_out=out_t[:, 0:1]).then_inc(dve_sem, 1)
            v.wait_ge(dve_sem, 6)
            v.scalar_tensor_tensor(
                wt["scr4"][:, :], wt["W_"][:, :], 1.0, pg["Are"][:, :],
                mul, mul, accum_out=out_t[:, 3:4]).then_inc(dve_sem, 1)
            v.wait_ge(pe_sem, 10)
            v.scalar_tensor_tensor(
                wt["scr3"][:, :], wt["U_"][:, :], 1.0, pg["Aim"][:, :],
                mul, mul, accum_out=out_t[:, 2:3]).then_inc(dve_sem, 1)
            v.scalar_tensor_tensor(
                wt["scr2"][:, :], wt["W_"][:, :], 1.0, pg["Aim"][:, :],
                mul, mul, accum_out=out_t[:, 1:2]).then_inc(dve_sem, 1)

        _gp_stream()
        _sync_stream()
        _act_stream()
        _pe_stream()
        _dve_stream()
        # no explicit epilogue: the NRT postamble quiesces DMA rings

    nc.finalize()
    _PROGRAM_CACHE["prog"] = nc
    return nc


def kernel(A_real, A_imag, _collect=None):
    from concourse.bass_utils import run_bass_kernel_spmd

    A = np.asarray(A_real, np.float64) + 1j * np.asarray(A_imag, np.float64)
    nc = _build_program()
    in_maps = [_build_core_tables(A, c) for c in range(N_CORES)]

    kwargs = dict(_collect or {})
    res = run_bass_kernel_spmd(nc, in_maps, core_ids=list(range(N_CORES)), **kwargs)
    if _collect is not None:
        _collect["results"] = res

    total = np.complex128(0)
    for r in res.results:
        o = np.asarray(r["out"], np.float64).reshape(4)
        total += (o[0] - o[1]) + 1j * (o[2] + o[3])

    perm = total * 2.0 * (2.0 ** (1 - N))
    ans = (perm.conjugate() * perm).real
    return np.asarray(ans, np.float32)


# revision 4
# speedup vs baseline: 1.1927x; 1.1927x over previous
r"""Boson-sampling probability |Perm(A)|^2 via Glynn's formula on 8 Trainium2 cores.

Math
----
perm(A) = 2^(1-n) * sum_{d in {-1,+1}^n} (prod_i d_i) * prod_j (sum_i d_i A_ij), n=20.
Terms for d and -d are equal, so enumerate d_19 = -1 only and double.

Sign-bit allocation for the remaining 19 bits:
  bits 0..8   -> free axis f (512)       [same on every core]
  bits 9..15  -> partition axis p (128)  [same on every core]
  bits 16..18 -> core c (8)

Row vector V_j(p,f,c) = Cp_c[p,j] + Cf[f,j] with
  Cp_c[p,j] = sum_{i=9..15} d_i(p) A[i,j] + sum_{i=16..18} d_i(c) A[i,j] - A[19,j]
  Cf[f,j]   = sum_{i=0..8} d_i(f) A[i,j]

Split the j-product into groups GA=0..6, GB=7..13, GC=14..19. Each group
product expands over subsets T of the group:
  PG[p,f] = sum_T (prod_{j in T} Cp[p,j]) * (prod_{j in G\T} Cf[f,j])
a bilinear form of rank 2^|G| -> computed on TensorE as fp16 matmuls with
PSUM accumulation. VectorE combines P = PA*PB*PC (complex) and reduces
over f with fused scalar_tensor_tensor accumulate ops; the (128,4)
per-core partials are summed on host in float64.

Schedule (from perfetto analysis; v1 @23.5us -> this @~22.1us measured):
 - Table loads spread over all three DGE paths so the SDMA queues
   round-robin at full aggregate share under 8-core HBM contention
   (~265 GB/s/core active streaming): B (chain gate) alone on the SP
   HWDGE ring, C then A0 on the ACT ring, A1 via GPSIMD SWDGE. One
   packed [128, 5*768] fp16 DRAM tensor per core, partition-contiguous,
   one descriptor per partition per transfer.
 - 26 F=256 dummy matmuls bridge the PE HAM activity window across the
   ~6us DMA wait, so the real matmuls run at the ramped clock
   (216ns/512-col matmul instead of 427/585).
 - Evictions split ACT/DVE: ACT copies sPCre (after the Cre matmul
   alone), sPCim, sPBim; the DVE casts sPBre itself so the combine
   chain launches the moment the Bre accumulation closes. DVE is the
   critical-path engine: cast + 4 muls + sub/add + 4 fused-reduce STTs
   (~5.0us busy, all 1x/2x bound).
 - ACT warms its activation table with a dummy copy at t=0 so the
   1.3us ACT_TABLE_LOAD overlaps the DMA head.
 - The PE partition-reduces the four [128,1] accumulator columns with a
   ones-lhsT matmul so the output store is a single 16B descriptor.
 - Fixed, non-removable overhead measured on this runtime: ~1.2us bass
   init barrier + ~0.7us descgen + ~0.8us doorbell-to-first-byte +
   ~6.7us end-of-NEFF semaphore-ceremony (53 EVENT_SEMAPHOREs/engine in
   lockstep, count invariant to kernel semaphore usage) + ~1.4us output
   DMA round trip.
"""

import numpy as np

N = 20
N_CORES = 8
F = 512           # free size (bits 0..8)
P = 128           # partitions (bits 9..15)
GA = list(range(0, 7))
GB = list(range(7, 14))
GC = list(range(14, 20))
WCH = 2 * P + F   # 768 columns per chunk: [lhsT_re | lhsT_im | vtab]
# chunk order in the packed table / SBUF: C(1), B(2), A(2)
NCH = 5

_PROGRAM_CACHE = {}


def _signs(count, nbits):
    v = np.arange(count, dtype=np.int64)[:, None]
    return (((v >> np.arange(nbits)) & 1) * 2.0 - 1.0)  # (count, nbits) float64


def _subset_prods(C):
    """C: (nvals, g) complex128 -> (2^g, nvals); row T = prod_{k: bit k of T} C[:, k]."""
    out = np.ones((1, C.shape[0]), np.complex128)
    for k in range(C.shape[1]):
        out = np.concatenate([out, out * C[None, :, k]], axis=0)
    return out


def _pack_group(U, V):
    """Interleave re/im rows for the paired-contraction matmul layout.

    One shared V table streams through two matmuls; the re/im arithmetic is
    carried by two lhsT variants (contraction rows m = 2T + c):
      vtab[2T]   = Re V[T],  vtab[2T+1]   = Im V[T]
      lhs_re[2T] = Re U[T],  lhs_re[2T+1] = -Im U[T]   (-> PG_re)
      lhs_im[2T] = Im U[T],  lhs_im[2T+1] =  Re U[T]   (-> PG_im)
    """
    nT = U.shape[0]
    lre = np.empty((2 * nT, U.shape[1]), np.float32)
    lre[0::2] = U.real
    lre[1::2] = -U.imag
    lim = np.empty((2 * nT, U.shape[1]), np.float32)
    lim[0::2] = U.imag
    lim[1::2] = U.real
    vtab = np.empty((2 * nT, V.shape[1]), np.float32)
    vtab[0::2] = V.real
    vtab[1::2] = V.imag
    return lre, lim, vtab


def _build_core_tables(A, core):
    """Host tables for one core, packed as one (128, 5*768) fp16 array with
    per-partition-contiguous chunk layout [C | B0 | B1 | A0 | A1]."""
    f_signs = _signs(F, 9)
    p_signs = _signs(P, 7)
    c_signs = _signs(N_CORES, 3)
    par_f = np.prod(f_signs, axis=1)
    par_p = np.prod(p_signs, axis=1)
    par_c = np.prod(c_signs[core])

    Cf = f_signs @ A[0:9, :]                                         # (512, 20)
    Cp = p_signs @ A[9:16, :] + (c_signs[core] @ A[16:19, :] - A[19, :])[None, :]

    chunks = {}
    for name, G in (("A", GA), ("B", GB), ("C", GC)):
        U = _subset_prods(Cp[:, G])          # (2^g, 128)
        VV = _subset_prods(Cf[:, G])         # (2^g, 512)
        V = VV[::-1]                         # complement subset: T -> 2^g-1-T
        if name == "A":
            # fold full parity: par_p(p) * par_f(f) * par_c * (-1 for d19)
            U = U * (par_p[None, :] * (-par_c))
            V = V * par_f[None, :]
        lre, lim, vtab = _pack_group(U, V)
        nchunks = lre.shape[0] // 128
        packed = np.concatenate([lre, lim, vtab], axis=1)  # (2^g*2, 768)
        chunks[name] = packed.reshape(nchunks, 128, WCH).astype(np.float16)
    # partition-major packing: [B0 B1 | C | A0 A1]; B rides the SP ring
    # (chain gate), C the ACT ring (small, early), A0/A1 ACT+GP (needed last)
    tab = np.concatenate(
        [chunks["B"][0], chunks["B"][1], chunks["C"][0],
         chunks["A"][0], chunks["A"][1]], axis=1)          # (128, 3840)
    return {"tab": np.ascontiguousarray(tab)}


def _build_program():
    if "prog" in _PROGRAM_CACHE:
        return _PROGRAM_CACHE["prog"]

    from contextlib import ExitStack
    from concourse import bass, mybir

    f32 = mybir.dt.float32
    f16 = mybir.dt.float16
    mul = mybir.AluOpType.mult
    nc = bass.Bass()

    W = NCH * WCH
    tab_dram = nc.declare_dram_parameter("tab", [128, W], f16, isOutput=False)
    out_dram = nc.declare_dram_parameter("out", [1, 4], f32, isOutput=True)

    es = ExitStack()
    with es:
        dsem = [es.enter_context(nc.semaphore(f"dma{i}")) for i in range(4)]
        pe_sem = es.enter_context(nc.semaphore("pe_sem"))
        act_sem = es.enter_context(nc.semaphore("act_sem"))
        dve_sem = es.enter_context(nc.semaphore("dve_sem"))
        gp_sem = es.enter_context(nc.semaphore("gp_sem"))

        sb = es.enter_context(nc.sbuf_tensor("sb_tab", [128, W], f16))
        names = ["sPCre", "sPCim", "sPBre", "sPBim",
                 "t1", "t2", "t3", "t4", "U_", "W_",
                 "scr1", "scr2", "scr3", "scr4"]
        wt = {n: es.enter_context(nc.sbuf_tensor(n, [P, F], f16)) for n in names}
        out_t = es.enter_context(nc.sbuf_tensor("out_t", [P, 4], f32))
        out_s = es.enter_context(nc.sbuf_tensor("out_s", [1, 4], f32))
        dummy = es.enter_context(nc.sbuf_tensor("actwarm", [P, 2], f32))
        pewarm = es.enter_context(nc.sbuf_tensor("pewarm", [P, 3 * P], f16))
        ones_ap = nc.const_aps.aps[(f32, 1.0)]
        pg = {}
        for g in ("A", "B", "C"):
            for comp in ("re", "im"):
                pg[g + comp] = es.enter_context(
                    nc.psum_tensor("pg" + g + comp, [P, F], f32))
        pgwarm = es.enter_context(nc.psum_tensor("pgwarm", [P, F], f32))

        # chunk column offsets in sb/tab: C, B0, B1, A0, A1
        def ap_lhs(ci, comp):
            lo = ci * WCH + (0 if comp == "re" else P)
            return sb[:, lo:lo + P]

        def ap_rhs(ci):
            lo = ci * WCH + 2 * P
            return sb[:, lo:lo + F]

        def _gp_stream():
            gp = nc.gpsimd
            # A1 via SWDGE (needed last; Pool descgen runs off-critical-path)
            gp.dma_start(sb[:, 4 * WCH:5 * WCH], tab_dram[:, 4 * WCH:5 * WCH]).then_inc(dsem[3], 16)

        def _sync_stream():
            sync = nc.sync
            # B (chain gate) alone on the SP ring: earliest big transfer
            sync.dma_start(sb[:, 0:2 * WCH], tab_dram[:, 0:2 * WCH]).then_inc(dsem[0], 16)

        def _pe_stream():
            pe = nc.tensor
            # Dummy matmuls during the DMA wait keep the PE HAM activity
            # window full, so the real matmuls run at the 2.4GHz p-state
            # (213ns per F=512 matmul) instead of 1.2GHz.
            # F=256 dummies bridge the PE HAM activity window across the DMA
            # wait, so the real matmuls run at the ramped clock (engages
            # ~5.7-6us after busy-start; tables land ~6.3-6.9us in). They
            # start immediately at body start: pewarm is read uninitialized,
            # which is safe -- any NaNs land only in the pgwarm scratch bank,
            # which the final partition-reduce resets with start=True.
            for _ in range(26):
                pe.matmul(pgwarm[:, 0:2 * P], pewarm[:, 0:P], pewarm[:, P:3 * P],
                          start=True, stop=True)

            # pe_sem: PC done at 2, PBre at 4, PBim at 6, PAre at 8, PAim at 10
            # chunk order in sb: B0=0, B1=1, C=2, A0=3, A1=4
            def mm(g, comp, ci, first, last, w=None):
                if w is not None:
                    pe.wait_ge(dsem[w], 16)
                pe.matmul(
                    pg[g + comp][:, :], ap_lhs(ci, comp), ap_rhs(ci),
                    start=first, stop=last,
                ).then_inc(pe_sem, 1)
            mm("C", "re", 2, True, True, w=1)
            mm("C", "im", 2, True, True)
            mm("B", "re", 0, True, False, w=0)
            mm("B", "re", 1, False, True)
            mm("B", "im", 0, True, False)
            mm("B", "im", 1, False, True)
            mm("A", "re", 3, True, False, w=2)
            mm("A", "re", 4, False, True, w=3)
            mm("A", "im", 3, True, False)
            mm("A", "im", 4, False, True)
            # partition-reduce the 4 accumulator columns so the store is one
            # 16B descriptor instead of 128
            pe.wait_ge(dve_sem, 10)
            pe.matmul(pgwarm[0:1, 0:4], ones_ap, out_t[:, :],
                      start=True, stop=True).then_inc(pe_sem, 1)

        def _act_stream():
            act = nc.scalar
            # C (small, needed first) then A0 on the ACT HWDGE ring
            act.dma_start(sb[:, 2 * WCH:3 * WCH], tab_dram[:, 2 * WCH:3 * WCH]).then_inc(dsem[1], 16)
            act.dma_start(sb[:, 3 * WCH:4 * WCH], tab_dram[:, 3 * WCH:4 * WCH]).then_inc(dsem[2], 16)
            # dummy first op: pulls the 1.3us ACT table load into the DMA head
            act.copy(dummy[:, 1:2], dummy[:, 0:1])
            # PSUM->SBUF fp16 evictions (PC, PBim); DVE casts sPBre itself.
            # sPCre only needs the Cre matmul (pe>=1), so it starts while
            # Cim is still streaming.
            act.wait_ge(pe_sem, 1)
            act.copy(wt["sPCre"][:, :], pg["Cre"][:, :]).then_inc(act_sem, 1)
            act.wait_ge(pe_sem, 2)
            act.copy(wt["sPCim"][:, :], pg["Cim"][:, :]).then_inc(act_sem, 1)
            act.wait_ge(pe_sem, 6)
            act.copy(wt["sPBim"][:, :], pg["Bim"][:, :]).then_inc(act_sem, 1)
            # evict the PE's 4-column partition-reduction, store via a single
            # 16B descriptor
            act.wait_ge(pe_sem, 11)
            act.copy(out_s[:, :], pgwarm[0:1, 0:4])
            act.dma_start(out_dram[:], out_s[:, :]).then_inc(dsem[0], 16)

        def _dve_stream():
            v = nc.vector
            # M = PC*PB in fp16 2x mode; fused 1x dot-products against PA in
            # PSUM. Standalone self-waits make same-engine RAW explicit.
            v.memset(dummy[:, 0:1], 0.0)
            # self-evict PBre (2x-mode fp16 cast) so the chain isn't queued
            # behind ACT's eviction backlog
            v.wait_ge(pe_sem, 4)
            v.tensor_copy(wt["sPBre"][:, :], pg["Bre"][:, :])
            v.wait_ge(act_sem, 1)
            v.tensor_mul(wt["t1"][:, :], wt["sPCre"][:, :], wt["sPBre"][:, :]).then_inc(dve_sem, 1)
            v.wait_ge(act_sem, 2)
            v.tensor_mul(wt["t4"][:, :], wt["sPCim"][:, :], wt["sPBre"][:, :]).then_inc(dve_sem, 1)
            v.wait_ge(act_sem, 3)
            v.tensor_mul(wt["t2"][:, :], wt["sPCim"][:, :], wt["sPBim"][:, :]).then_inc(dve_sem, 1)
            v.tensor_mul(wt["t3"][:, :], wt["sPCre"][:, :], wt["sPBim"][:, :]).then_inc(dve_sem, 1)
            v.wait_ge(dve_sem, 3)
            v.tensor_sub(wt["U_"][:, :], wt["t1"][:, :], wt["t2"][:, :]).then_inc(dve_sem, 1)
            v.wait_ge(dve_sem, 4)
            v.tensor_add(wt["W_"][:, :], wt["t3"][:, :], wt["t4"][:, :]).then_inc(dve_sem, 1)
            # out cols: 0 = sum U*PAre, 1 = sum W*PAim, 2 = sum U*PAim,
            # 3 = sum W*PAre ; host computes re = c0-c1, im = c2+c3.
            v.wait_ge(pe_sem, 8)
            v.wait_ge(dve_sem, 5)
            v.scalar_tensor_tensor(
                wt["scr1"][:, :], wt["U_"][:, :], 1.0, pg["Are"][:, :],
                mul, mul, accum_out=out_t[:, 0:1]).then_inc(dve_sem, 1)
            v.wait_ge(dve_sem, 6)
            v.scalar_tensor_tensor(
                wt["scr4"][:, :], wt["W_"][:, :], 1.0, pg["Are"][:, :],
                mul, mul, accum_out=out_t[:, 3:4]).then_inc(dve_sem, 1)
            v.wait_ge(pe_sem, 10)
            v.scalar_tensor_tensor(
                wt["scr3"][:, :], wt["U_"][:, :], 1.0, pg["Aim"][:, :],
                mul, mul, accum_out=out_t[:, 2:3]).then_inc(dve_sem, 1)
            v.scalar_tensor_tensor(
                wt["scr2"][:, :], wt["W_"][:, :], 1.0, pg["Aim"][:, :],
                mul, mul, accum_out=out_t[:, 1:2]).then_inc(dve_sem, 1)

        _gp_stream()
        _sync_stream()
        _act_stream()
        _pe_stream()
        _dve_stream()
        # no explicit epilogue: the NRT postamble quiesces DMA rings

    nc.finalize()
    _PROGRAM_CACHE["prog"] = nc
    return nc


def kernel(A_real, A_imag, _collect=None):
    from concourse.bass_utils import run_bass_kernel_spmd

    A = np.asarray(A_real, np.float64) + 1j * np.asarray(A_imag, np.float64)
    nc = _build_program()
    in_maps = [_build_core_tables(A, c) for c in range(N_CORES)]

    kwargs = dict(_collect or {})
    res = run_bass_kernel_spmd(nc, in_maps, core_ids=list(range(N_CORES)), **kwargs)
    if _collect is not None:
        _collect["results"] = res

    total = np.complex128(0)
    for r in res.results:
        o = np.asarray(r["out"], np.float64).reshape(4)
        total += (o[0] - o[1]) + 1j * (o[2] + o[3])

    perm = total * 2.0 * (2.0 ** (1 - N))
    ans = (perm.conjugate() * perm).real
    return np.asarray(ans, np.float32)
